# revision 1
# baseline (speedup 1.0000x reference)
"""GCN 2-layer encoder for Trainium2, 8-core SPMD.

Strategy (see problem spec):
- Nodes padded to N_PAD = 8 * NB_C * 128, sharded by contiguous 128-node
  blocks across 8 cores (NB_C blocks per core).
- GCNConv linearity: layer 1 propagates raw x (128 feats) then applies W1;
  layer 2 applies W2 then propagates the 128-feat result.
- Edge weights dinv[src]*dinv[dst] are factored: src side folded into a
  per-edge weight on the one-hot scatter matrix (layer 1) / pre-scaled table
  rows (layer 2), dst side applied per-block after aggregation.
- Scatter-add is a TensorE matmul against a per-chunk one-hot matrix built
  on DVE with is_equal; edges are host-sorted by (dst block, src half) and
  chunked 128 at a time (contraction dim).
- Gather uses gpsimd.dma_gather from bf16 tables (256B rows); node space is
  split in two halves so indices fit int16.
- Between layers the per-core t~ = dinv * (h1 @ W2) shards are AllGathered.
"""

import numpy as np
import ml_dtypes

import concourse.bass as bass
import concourse.bacc as bacc
import concourse.mybir as mybir
import concourse.tile as tile

BF16 = ml_dtypes.bfloat16
BLK = 128
NC_CORES = 8


def make_cfg(n_nodes, nb_c):
    """nb_c: blocks per core (must be even)."""
    assert nb_c % 2 == 0
    core_n = nb_c * BLK
    n_pad = NC_CORES * core_n
    assert n_pad >= n_nodes
    return dict(
        N=n_nodes,
        NB_C=nb_c,
        CORE_N=core_n,
        N_PAD=n_pad,
        H_SPLIT=n_pad // 2,
        NGRP=nb_c // 2,
    )


def host_prep(x, edge_index, W1, b1, W2, b2, cfg):
    """Index/layout preprocessing. Returns (in_maps, meta)."""
    N, D = x.shape
    assert N == cfg["N"] and D == 128
    N_PAD, H_SPLIT = cfg["N_PAD"], cfg["H_SPLIT"]
    NB_C, CORE_N, NGRP = cfg["NB_C"], cfg["CORE_N"], cfg["NGRP"]

    src = edge_index[0].astype(np.int64)
    dst = edge_index[1].astype(np.int64)
    deg = np.bincount(dst, minlength=N).astype(np.float64) + 1.0
    dinv_pad = np.ones(N_PAD, np.float32)
    dinv_pad[:N] = (1.0 / np.sqrt(deg)).astype(np.float32)

    # self loops as plain edges
    src = np.concatenate([src, np.arange(N, dtype=np.int64)])
    dst = np.concatenate([dst, np.arange(N, dtype=np.int64)])

    blk = dst >> 7
    half = (src >= H_SPLIT).astype(np.int64)
    order = np.lexsort((half, blk))
    src, dst, half, blk = src[order], dst[order], half[order], blk[order]

    NBLK = NC_CORES * NB_C
    key = blk * 2 + half
    counts = np.bincount(key, minlength=NBLK * 2).reshape(NBLK, 2)
    starts = np.zeros(NBLK * 2 + 1, np.int64)
    np.cumsum(counts.reshape(-1), out=starts[1:])
    CH = int(np.ceil(counts.max() / BLK))

    CPG = 4 * CH                      # chunk columns per group
    CHUNK_COLS = NGRP * CPG
    NIDX_CALL = 2 * CH * BLK          # idxs per gather call (2 blocks, 1 half)
    IDXC = NIDX_CALL // 16            # idx sbuf columns per call
    NCALL = 2 * NGRP

    idx_np = np.zeros((NC_CORES, 128, NCALL * IDXC), np.int16)
    dstloc_np = np.full((NC_CORES, 128, CHUNK_COLS), -1.0, np.float32)
    srcw_np = np.zeros((NC_CORES, 128, CHUNK_COLS), np.float32)

    for c in range(NC_CORES):
        for g in range(NGRP):
            for h in (0, 1):
                idx_call = np.zeros(NIDX_CALL, np.int16)
                for j in (0, 1):
                    b = c * NB_C + 2 * g + j
                    s0, cnt = starts[b * 2 + h], counts[b, h]
                    e_src = src[s0 : s0 + cnt]
                    e_dst = dst[s0 : s0 + cnt]
                    idx_call[j * CH * BLK : j * CH * BLK + cnt] = (
                        e_src - h * H_SPLIT
                    ).astype(np.int16)
                    dl = np.full(CH * BLK, -1.0, np.float32)
                    dl[:cnt] = (e_dst - b * BLK).astype(np.float32)
                    sw = np.zeros(CH * BLK, np.float32)
                    sw[:cnt] = dinv_pad[e_src]
                    c0 = g * CPG + h * 2 * CH + j * CH
                    dstloc_np[c, :, c0 : c0 + CH] = dl.reshape(CH, BLK).T
                    srcw_np[c, :, c0 : c0 + CH] = sw.reshape(CH, BLK).T
                ci = 2 * g + h
                wrapped = idx_call.reshape(IDXC, 16).T  # [16, IDXC]
                idx_np[c, :, ci * IDXC : (ci + 1) * IDXC] = np.tile(wrapped, (8, 1))

    x_bf = np.zeros((N_PAD, 128), BF16)
    x_bf[:N] = x.astype(BF16)

    iota = np.tile(np.arange(128, dtype=np.float32), (128, 1)).astype(BF16)
    ident = np.eye(128, dtype=np.float32).astype(BF16)
    b1c = np.ascontiguousarray(b1.reshape(2, 128).T.astype(np.float32))
    b2r = np.ascontiguousarray(b2.reshape(1, 128).astype(np.float32))

    in_maps = []
    for c in range(NC_CORES):
        dslice = dinv_pad[c * CORE_N : (c + 1) * CORE_N]
        in_maps.append(
            {
                "x_bf": x_bf,
                "W1": np.ascontiguousarray(W1.astype(np.float32)),
                "b1c": b1c,
                "W2": np.ascontiguousarray(W2.astype(np.float32)),
                "b2r": b2r,
                "iota": iota,
                "ident": ident,
                "idx": np.ascontiguousarray(idx_np[c]),
                "dstloc": np.ascontiguousarray(dstloc_np[c].astype(BF16)),
                "srcw": np.ascontiguousarray(srcw_np[c].astype(BF16)),
                "dinvrow": np.ascontiguousarray(dslice.reshape(1, CORE_N)),
                "dinvcol": np.ascontiguousarray(dslice.reshape(NB_C, BLK).T),
            }
        )
    meta = dict(CH=CH, CPG=CPG, CHUNK_COLS=CHUNK_COLS, IDXC=IDXC, NCALL=NCALL,
                NIDX_CALL=NIDX_CALL)
    return in_maps, meta


def build_nc(cfg, meta):
    N_PAD, H_SPLIT = cfg["N_PAD"], cfg["H_SPLIT"]
    NB_C, CORE_N, NGRP = cfg["NB_C"], cfg["CORE_N"], cfg["NGRP"]
    CH, CPG, CHUNK_COLS, IDXC = meta["CH"], meta["CPG"], meta["CHUNK_COLS"], meta["IDXC"]
    NIDX_CALL = meta["NIDX_CALL"]
    dt = mybir.dt
    BF, F32, I16 = dt.bfloat16, dt.float32, dt.int16
    EQ = mybir.AluOpType.is_equal
    MUL = mybir.AluOpType.mult

    nc = bacc.Bacc(None, num_devices=NC_CORES)

    x_bf = nc.dram_tensor("x_bf", [N_PAD, 128], BF, kind="ExternalInput")
    W1 = nc.dram_tensor("W1", [128, 256], F32, kind="ExternalInput")
    b1c = nc.dram_tensor("b1c", [128, 2], F32, kind="ExternalInput")
    W2 = nc.dram_tensor("W2", [256, 128], F32, kind="ExternalInput")
    b2r = nc.dram_tensor("b2r", [1, 128], F32, kind="ExternalInput")
    iota = nc.dram_tensor("iota", [128, 128], BF, kind="ExternalInput")
    ident = nc.dram_tensor("ident", [128, 128], BF, kind="ExternalInput")
    idx = nc.dram_tensor("idx", [128, 2 * NGRP * IDXC], I16, kind="ExternalInput")
    dstloc = nc.dram_tensor("dstloc", [128, CHUNK_COLS], BF, kind="ExternalInput")
    srcw = nc.dram_tensor("srcw", [128, CHUNK_COLS], BF, kind="ExternalInput")
    dinvrow = nc.dram_tensor("dinvrow", [1, CORE_N], F32, kind="ExternalInput")
    dinvcol = nc.dram_tensor("dinvcol", [128, NB_C], F32, kind="ExternalInput")
    out = nc.dram_tensor("out", [CORE_N, 128], F32, kind="ExternalOutput")

    t_shard = nc.dram_tensor("t_shard", [CORE_N, 128], BF)
    t_full = nc.dram_tensor("t_full", [N_PAD, 128], BF, addr_space="Shared")

    with tile.TileContext(nc) as tc:
        with (
            tc.tile_pool(name="const", bufs=1) as cp,
            tc.tile_pool(name="gat", bufs=3) as gatp,
            tc.tile_pool(name="oh", bufs=3) as ohp,
            tc.tile_pool(name="sb", bufs=3) as sbp,
            tc.tile_pool(name="psA", bufs=2, space="PSUM") as psA,
            tc.tile_pool(name="psB", bufs=2, space="PSUM") as psB,
            tc.tile_pool(name="psC", bufs=2, space="PSUM") as psC,
            tc.tile_pool(name="psD", bufs=2, space="PSUM") as psD,
        ):
            # ---------- resident constants ----------
            idx_sb = cp.tile([128, 2 * NGRP * IDXC], I16)
            nc.sync.dma_start(idx_sb[:], idx[:, :])
            dstloc_sb = cp.tile([128, CHUNK_COLS], BF)
            nc.sync.dma_start(dstloc_sb[:], dstloc[:, :])
            srcw_sb = cp.tile([128, CHUNK_COLS], BF)
            nc.sync.dma_start(srcw_sb[:], srcw[:, :])
            iota_sb = cp.tile([128, 128], BF)
            nc.sync.dma_start(iota_sb[:], iota[:, :])
            ident_sb = cp.tile([128, 128], BF)
            nc.sync.dma_start(ident_sb[:], ident[:, :])
            W1_sb = cp.tile([128, 256], F32)
            nc.sync.dma_start(W1_sb[:], W1[:, :])
            W2a_sb = cp.tile([128, 128], F32)
            nc.sync.dma_start(W2a_sb[:], W2[0:128, :])
            W2b_sb = cp.tile([128, 128], F32)
            nc.sync.dma_start(W2b_sb[:], W2[128:256, :])
            b1_sb = cp.tile([128, 2], F32)
            nc.sync.dma_start(b1_sb[:], b1c[:, :])
            b2b_sb = cp.tile([128, 128], F32)
            nc.sync.dma_start(b2b_sb[:], b2r[0:1, :].to_broadcast([128, 128]))
            dinvrow_sb = cp.tile([128, CORE_N], F32)
            nc.sync.dma_start(dinvrow_sb[:], dinvrow[0:1, :].to_broadcast([128, CORE_N]))
            dinvcol_sb = cp.tile([128, NB_C], F32)
            nc.sync.dma_start(dinvcol_sb[:], dinvcol[:, :])

            def gather_group(g, table):
                gat = gatp.tile([128, CPG * 128], BF)
                gat3 = gat[:].rearrange("p (c f) -> p c f", f=128)
                for h in (0, 1):
                    ci = 2 * g + h
                    lo, hi = (0, H_SPLIT) if h == 0 else (H_SPLIT, N_PAD)
                    nc.gpsimd.dma_gather(
                        out_ap=gat3[:, h * 2 * CH : (h + 1) * 2 * CH, :],
                        in_ap=table[lo:hi, :],
                        idxs_ap=idx_sb[:, ci * IDXC : (ci + 1) * IDXC],
                        num_idxs=NIDX_CALL,
                        num_idxs_reg=NIDX_CALL,
                        elem_size=128,
                    )
                return gat, gat3

            def onehot_group(g, weighted):
                oh = ohp.tile([128, CPG * 128], BF)
                oh3 = oh[:].rearrange("p (c f) -> p c f", f=128)
                dl = dstloc_sb[:, g * CPG : (g + 1) * CPG]
                nc.vector.tensor_tensor(
                    out=oh3,
                    in0=dl[:, :, None].to_broadcast([128, CPG, 128]),
                    in1=iota_sb[:][:, None, :].to_broadcast([128, CPG, 128]),
                    op=EQ,
                )
                if weighted:
                    sw = srcw_sb[:, g * CPG : (g + 1) * CPG]
                    nc.vector.tensor_tensor(
                        out=oh3,
                        in0=oh3,
                        in1=sw[:, :, None].to_broadcast([128, CPG, 128]),
                        op=MUL,
                    )
                return oh, oh3

            def block_chunks(j):
                # chunk indices (within group tile) belonging to block j of the pair
                return [h * 2 * CH + j * CH + k for h in (0, 1) for k in range(CH)]

            # ---------- layer 1 ----------
            for g in range(NGRP):
                gat, gat3 = gather_group(g, x_bf)
                oh, oh3 = onehot_group(g, weighted=True)
                for j in (0, 1):
                    b = 2 * g + j
                    chunks = block_chunks(j)
                    psumA = psA.tile([128, 128], F32)
                    for k, ch in enumerate(chunks):
                        nc.tensor.matmul(
                            out=psumA[:],
                            lhsT=gat3[:, ch, :],
                            rhs=oh[:, ch * 128 : (ch + 1) * 128],
                            start=(k == 0),
                            stop=(k == len(chunks) - 1),
                        )
                    drow = dinvrow_sb[:, b * 128 : (b + 1) * 128]
                    agg1T = sbp.tile([128, 128], F32)
                    nc.vector.tensor_tensor(out=agg1T[:], in0=psumA[:], in1=drow, op=MUL)
                    psumB = psB.tile([128, 256], F32)
                    nc.tensor.matmul(
                        out=psumB[:, 0:128], lhsT=W1_sb[:, 0:128], rhs=agg1T[:],
                        start=True, stop=True,
                    )
                    nc.tensor.matmul(
                        out=psumB[:, 128:256], lhsT=W1_sb[:, 128:256], rhs=agg1T[:],
                        start=True, stop=True,
                    )
                    h1 = sbp.tile([128, 256], F32)
                    nc.scalar.activation(
                        out=h1[:, 0:128], in_=psumB[:, 0:128],
                        func=mybir.ActivationFunctionType.Relu, bias=b1_sb[:, 0:1],
                    )
                    nc.scalar.activation(
                        out=h1[:, 128:256], in_=psumB[:, 128:256],
                        func=mybir.ActivationFunctionType.Relu, bias=b1_sb[:, 1:2],
                    )
                    psumC = psC.tile([128, 128], F32)
                    nc.tensor.matmul(
                        out=psumC[:], lhsT=W2a_sb[:], rhs=h1[:, 0:128],
                        start=True, stop=False,
                    )
                    nc.tensor.matmul(
                        out=psumC[:], lhsT=W2b_sb[:], rhs=h1[:, 128:256],
                        start=False, stop=True,
                    )
                    tT_bf = sbp.tile([128, 128], BF)
                    nc.vector.tensor_tensor(out=tT_bf[:], in0=psumC[:], in1=drow, op=MUL)
                    psumD = psD.tile([128, 128], BF)
                    nc.tensor.transpose(out=psumD[:], in_=tT_bf[:], identity=ident_sb[:])
                    t_blk = sbp.tile([128, 128], BF)
                    nc.vector.tensor_copy(out=t_blk[:], in_=psumD[:])
                    nc.sync.dma_start(out=t_shard[b * 128 : (b + 1) * 128, :], in_=t_blk[:])

            # ---------- exchange ----------
            nc.gpsimd.collective_compute(
                "AllGather",
                mybir.AluOpType.bypass,
                replica_groups=[list(range(NC_CORES))],
                ins=[t_shard.ap().opt()],
                outs=[t_full.ap().opt()],
            )

            # ---------- layer 2 ----------
            for g in range(NGRP):
                gat, gat3 = gather_group(g, t_full)
                oh, oh3 = onehot_group(g, weighted=False)
                for j in (0, 1):
                    b = 2 * g + j
                    chunks = block_chunks(j)
                    psumA = psA.tile([128, 128], F32)
                    for k, ch in enumerate(chunks):
                        nc.tensor.matmul(
                            out=psumA[:],
                            lhsT=oh[:, ch * 128 : (ch + 1) * 128],
                            rhs=gat3[:, ch, :],
                            start=(k == 0),
                            stop=(k == len(chunks) - 1),
                        )
                    out_blk = sbp.tile([128, 128], F32)
                    nc.vector.tensor_scalar(
                        out=out_blk[:], in0=psumA[:],
                        scalar1=dinvcol_sb[:, b : b + 1], scalar2=None,
                        op0=MUL,
                    )
                    nc.vector.tensor_tensor(out=out_blk[:], in0=out_blk[:], in1=b2b_sb[:], op=mybir.AluOpType.add)
                    nc.sync.dma_start(out=out[b * 128 : (b + 1) * 128, :], in_=out_blk[:])

    nc.finalize()
    return nc


def run_full(x, edge_index, W1, b1, W2, b2, nb_c=50, runner=None):
    """Build, run on 8 cores, return [N,128] float32 output."""
    from concourse.bass_utils import run_bass_kernel_spmd

    cfg = make_cfg(x.shape[0], nb_c)
    in_maps, meta = host_prep(x, edge_index, W1, b1, W2, b2, cfg)
    nc = build_nc(cfg, meta)
    if runner is None:
        res = run_bass_kernel_spmd(nc, in_maps, core_ids=list(range(NC_CORES)))
        results = res.results
    else:
        results = runner(nc, in_maps)
    outs = np.concatenate([results[c]["out"] for c in range(NC_CORES)], axis=0)
    return outs[: x.shape[0]]


# ---------------------------------------------------------------------------
# Self-contained harness entry point
# ---------------------------------------------------------------------------

_CACHE = {}


def kernel(x, edge_index, W1, b1, W2, b2):
    """Full-input GCN encoder on 8 NeuronCores. Returns [N, 128] float32."""
    from concourse.bass_utils import run_bass_kernel_spmd

    x = np.asarray(x)
    cfg = make_cfg(x.shape[0], 50)
    in_maps, meta = host_prep(
        x, np.asarray(edge_index), np.asarray(W1), np.asarray(b1),
        np.asarray(W2), np.asarray(b2), cfg,
    )
    key = (x.shape[0], meta["CH"])
    if key not in _CACHE:
        _CACHE[key] = build_nc(cfg, meta)
    nc = _CACHE[key]
    res = run_bass_kernel_spmd(nc, in_maps, core_ids=list(range(NC_CORES)))
    outs = np.concatenate(
        [res.results[c]["out"] for c in range(NC_CORES)], axis=0
    )
    return outs[: x.shape[0]].astype(np.float32)
